# revision 56
# baseline (speedup 1.0000x reference)
"""Causal self-attention (B=2, S=2048, D=1024, H=16) on 8 NeuronCores.

Sharding (per spec hint): data-parallel over batch (2 groups of 4 cores),
tensor-parallel over heads within a group (4 heads / core). Each core
computes Q/K/V projections for its 4 heads, causal attention, and a partial
output projection through its slice of Wo. The 4 partial [2048, 1024]
outputs per batch are summed on the host (unsharding step).

Per-core kernel layout notes:
  - bf16 operands everywhere on the PE (fp32 PSUM accumulation): halves DMA
    traffic and allows arbitrary moving sizes at full PE rate, enabling
    exact causal trimming of the score matmuls.
  - Projections run k-chunk-major inside each 512-query group so the PE
    consumes x.T chunks as their DMAs land; attention for query chunk c-1
    is emitted between projection groups c-1 and c so the scalar engine's
    exp stream (the attention bottleneck) starts ~30us before projections
    finish.
  - Scores are computed transposed, ST[k, q]; P@V is computed with P as
    the stationary operand per 128-query tile, producing O[q, d] directly
    with a fused ones-column denominator (col 64 of each head's V block).
    Softmax normalize is then a per-partition tensor_scalar multiply, and
    the two heads of a pair pack side-by-side in the free dim. A DMA
    transpose (XBAR) turns the normalized [q, d] tile into the [d, q]
    stationary tile the out-projection needs - no PE transposes, no
    partition-crossing copies.
  - A single 8-bank PSUM pool (tag "big" 2-bank entries x3, tag "o"
    1-bank entries x2) is shared by projections, scores, P@V and the
    out-projection so all phases can be in flight at once.
"""

import numpy as np
import ml_dtypes

import concourse.bass as bass
import concourse.mybir as mybir
import concourse.tile as tile
from concourse.bass_utils import run_bass_kernel_spmd

F32 = mybir.dt.float32
BF16 = mybir.dt.bfloat16
AF = mybir.ActivationFunctionType

B, S, D, H = 2, 2048, 1024, 16
DH = D // H              # 64
HL = 4                   # heads per core
CL = HL * DH             # 256 channels per core
G = 4                    # cores per batch group
SCALE = DH ** -0.5       # 0.125
NQC = S // 512           # 4 q-chunks of 512
NKT = S // 128           # 16 key tiles of 128


def _split_excess_waits(nc, max_waits=1):
    """walrus in this toolchain rejects instructions carrying more than
    `max_waits` sem waits; split the excess onto preceding same-engine
    NoOps (sound: waits are monotone >= conditions hoisted earlier on
    the same engine)."""
    n_split = 0
    for f in nc.m.functions:
        for bb in f.blocks:
            out = []
            for inst in bb.instructions:
                si = inst.sync_info
                waits = list(si.on_wait) if si is not None and si.on_wait else []
                if len(waits) > max_waits:
                    head, keep = waits[:-max_waits], waits[-max_waits:]
                    for ci, start in enumerate(range(0, len(head), max_waits)):
                        nop = mybir.InstNoOp(
                            name=f"{inst.name}_wsplit{ci}",
                            sync_info=mybir.SyncInfo(
                                on_wait=head[start:start + max_waits],
                                on_update=[],
                            ),
                            engine=inst.engine,
                            bass_nofuse=True,
                        )
                        out.append(nop)
                        n_split += 1
                    si.on_wait = keep
                out.append(inst)
            if n_split:
                bb.instructions.clear()
                for i in out:
                    bb.instructions.append(i)
    return n_split


def _build_nc(split_waits=True):
    nc = bass.Bass()
    xt_d = nc.dram_tensor("xt", [D, S], BF16, kind="ExternalInput")
    wq_d = nc.dram_tensor("wq", [D, CL], BF16, kind="ExternalInput")
    wk_d = nc.dram_tensor("wk", [D, CL], BF16, kind="ExternalInput")
    wv_d = nc.dram_tensor("wv", [D, CL], BF16, kind="ExternalInput")
    wo_d = nc.dram_tensor("wo", [128, 2, D], BF16, kind="ExternalInput")
    mask_d = nc.dram_tensor("mask", [128, 2, 128], BF16, kind="ExternalInput")
    y_d = nc.dram_tensor("y", [S, D], BF16, kind="ExternalOutput")

    with tile.TileContext(nc) as tc:
        with (
            tc.tile_pool(name="persist", bufs=1) as pp,
            tc.tile_pool(name="ps", bufs=1, space="PSUM") as ps,
            tc.tile_pool(name="ptp", bufs=24) as ptp,
            tc.tile_pool(name="onot", bufs=4) as onp,
            tc.tile_pool(name="otp", bufs=14) as otp,
            tc.tile_pool(name="ysp", bufs=8) as ysp,
            tc.tile_pool(name="rip", bufs=4) as rip,
        ):
            # ---- persistent SBUF tensors -------------------------------
            xt_sb = pp.tile([128, 8, S], BF16)
            wq_sb = pp.tile([128, 8, CL], BF16)
            wk_sb = pp.tile([128, 8, CL], BF16)
            wv_sb = pp.tile([128, 8, CL], BF16)
            wo_sb = pp.tile([128, 2, D], BF16)
            mask_sb = pp.tile([128, 2, 128], BF16)
            qt_sb = [pp.tile([128, S], BF16, name=f"qt{p}", tag=f"qt{p}")
                     for p in range(2)]
            kt_sb = [pp.tile([128, S], BF16, name=f"kt{p}", tag=f"kt{p}")
                     for p in range(2)]
            # V' per key-tile: 4x[64 v-cols + 1 ones-col]
            vp_sb = pp.tile([128, NKT, 4 * 65], BF16)
            wrm = pp.tile([128, 128], BF16)
            nc.gpsimd.memset(wrm[:], 0.0)
            for hl in range(4):
                nc.vector.memset(vp_sb[:, :, hl * 65 + 64:hl * 65 + 65], 1.0)

            # ---- input DMAs (SP queue, arrival-ordered) ----------------
            # group 0 only touches xt cols 0:512 (its queries AND its V row
            # block), so stream those quarters first, weights interleaved.
            xt_r = xt_d.rearrange("(a p) s -> p a s", p=128)
            wq_r = wq_d.rearrange("(a p) m -> p a m", p=128)
            wk_r = wk_d.rearrange("(a p) m -> p a m", p=128)
            wv_r = wv_d.rearrange("(a p) m -> p a m", p=128)
            # few, large DMAs (each dma_start costs ~625ns on the serial
            # HWDGE descriptor engine) but with small first chunks so the
            # first projection chain starts at ~2us.
            for kk in range(0, 8, 2):
                kks = slice(kk, kk + 2)
                nc.sync.dma_start(wq_sb[:, kks, :], wq_r[:, kks, :])
                nc.sync.dma_start(wk_sb[:, kks, :], wk_r[:, kks, :])
                nc.sync.dma_start(xt_sb[:, kks, 0:512], xt_r[:, kks, 0:512])
            nc.sync.dma_start(mask_sb[:], mask_d[:, :, :])
            nc.sync.dma_start(xt_sb[:, :, 512:1024], xt_r[:, :, 512:1024])
            nc.sync.dma_start(wv_sb[:], wv_r[:])
            nc.sync.dma_start(wo_sb[:], wo_d[:, :, :])
            nc.sync.dma_start(xt_sb[:, :, 1024:1536], xt_r[:, :, 1024:1536])
            nc.sync.dma_start(xt_sb[:, :, 1536:2048], xt_r[:, :, 1536:2048])

            # ---- PE filler queue ---------------------------------------
            # Projection work for group c+1 is queued in small chunks and
            # emitted between attention score matmuls, so the PE has ready
            # work while it waits for the scalar engine's exp stream (the
            # attention bottleneck) to drain score tiles.
            # one strict-FIFO queue of projection-chain chunks (labels
            # mark drain barriers); a chain's chunks stay contiguous so at
            # most one chain is mid-flight on the "w" ring at any time
            filler_ch = []       # (est, fn, label)
            filler_yp = []       # deferred out-projections: lowest priority

            epoch = [0]
            debt = [0.0]  # carried-over filler budget (amortizes chunk
                          # granularity so the ST stream never lags ACT)

            def emit_filler(budget, reserve_yp=0):
                debt[0] += budget
                while True:
                    if filler_ch:
                        q = filler_ch
                    elif len(filler_yp) > reserve_yp \
                            and filler_yp[0][2] <= epoch[0] - 8:
                        q = filler_yp
                    else:
                        debt[0] = min(debt[0], 2000.0)
                        return
                    if q[0][0] > debt[0]:
                        return
                    item = q.pop(0)
                    item[1]()
                    debt[0] -= item[0]

            def drain_until(label):
                while any(it[2] == label for it in filler_ch):
                    filler_ch.pop(0)[1]()

            def drain_yp():
                while filler_yp:
                    filler_yp.pop(0)[1]()

            # ---- projection chains (tag rings: see module docstring) ---
            def qk_chain(c, which):
                """Full 8-step QK chain for group c as filler chunks."""
                cslc = slice(c * 512, (c + 1) * 512)
                w_sb = wq_sb if which == "q" else wk_sb
                dst = qt_sb if which == "q" else kt_sb
                box = {}

                def step(k, p):
                    def go():
                        if k == 0:
                            box[p] = ps.tile([128, 512], F32, tag="w",
                                             bufs=2, name=f"pj{which}{c}{p}")
                        nc.tensor.matmul(
                            box[p][:], w_sb[:, k, p * 128:(p + 1) * 128],
                            xt_sb[:, k, cslc], start=(k == 0), stop=(k == 7))
                        if k == 7:
                            nc.vector.tensor_copy(dst[p][:, cslc], box[p][:])
                    return go
                return [(214, step(k, p), f"{which}{c}")
                        for k in range(8) for p in range(2)]

            def v_chains(c):
                """V chains for group c (4 row-tiles, j-sequential)."""
                box = {}

                def step(j, kk):
                    def go():
                        pair, jj = divmod(j, 2)
                        if jj == 0 and kk == 0:
                            box[pair] = ps.tile([128, 2, 256], F32, tag="w",
                                                bufs=2, name=f"pjv{c}{pair}")
                        st = 4 * c + j
                        for k in range(kk, kk + 2):
                            nc.tensor.matmul(
                                box[pair][:, jj, :],
                                xt_sb[:, k, st * 128:(st + 1) * 128],
                                wv_sb[:, k, :], start=(k == 0), stop=(k == 7))
                        if jj == 1 and kk == 6:
                            nc.vector.tensor_copy(
                                vp_sb[:, 4 * c + 2 * pair:4 * c + 2 * pair + 2, :]
                                .rearrange("p s (h e) -> p (s h) e", e=65)
                                [:, :, 0:64],
                                box[pair][:]
                                .rearrange("p s (h d) -> p (s h) d", d=64))
                    return go
                return [(214, step(j, kk), f"v{c}")
                        for j in range(4) for kk in (0, 2, 4, 6)]

            # ---- group 0: Q then K chains (V rides the filler queue) ----
            def proj_group0():
                psq = ps.tile([128, 2, 512], F32, tag="st", bufs=2)
                psk = ps.tile([128, 2, 512], F32, tag="st", bufs=2)
                for k in range(8):
                    first, last = k == 0, k == 7
                    for p in range(2):
                        nc.tensor.matmul(
                            psq[:, p, :], wq_sb[:, k, p * 128:(p + 1) * 128],
                            xt_sb[:, k, 0:512], start=first, stop=last)
                    for p in range(2):
                        nc.tensor.matmul(
                            psk[:, p, :], wk_sb[:, k, p * 128:(p + 1) * 128],
                            xt_sb[:, k, 0:512], start=first, stop=last)
                for p in range(2):
                    nc.scalar.copy(qt_sb[p][:, 0:512], psq[:, p, :])
                    nc.vector.tensor_copy(kt_sb[p][:, 0:512], psk[:, p, :])

            # ---- attention + out-projection for query chunk qc ---------
            y_r = y_d.rearrange("(a p) d -> p a d", p=128)

            def out_proj(OT, qt_g, on_act=False, endgame=False):
                ysb = ysp.tile([128, D], BF16, tag="ys")
                if endgame:
                    # score ring is free by now: 2-bank paired tiles let the
                    # four tail out-projections overlap their copies
                    ypp = ps.tile([128, 2, 512], F32, tag="st", bufs=2)
                for nch in range(2):
                    if endgame:
                        yp = ypp[:, nch, :]
                    else:
                        yp = ps.tile([128, 512], F32, tag="oy", bufs=2)
                    for kp in range(2):
                        nc.tensor.matmul(
                            yp, OT[:, kp, :],
                            wo_sb[:, kp, nch * 512:(nch + 1) * 512],
                            start=(kp == 0), stop=(kp == 1))
                    nsl = slice(nch * 512, (nch + 1) * 512)
                    if on_act and nch:  # endgame: split copies DVE/ACT
                        nc.scalar.copy(ysb[:, nsl], yp)
                    else:
                        nc.vector.tensor_copy(ysb[:, nsl], yp)
                    if endgame:  # idle SP queue, per-half DMAs
                        nc.sync.dma_start(y_r[:, qt_g, nsl], ysb[:, nsl])
                if not endgame:
                    nc.gpsimd.dma_start(y_r[:, qt_g, :], ysb[:])

            def attention(qc, late_k=()):
                qlo = qc * 512
                ktmax = 4 * (qc + 1)
                onot = [(onp.tile([128, 2, 128], BF16, tag="on", name=f"on{qt}"),
                         otp.tile([128, 2, 128], BF16, tag="ot", name=f"ot{qt}"))
                        for qt in range(4)]
                pts = {0: [], 1: []}

                def score(p, kt):
                    if kt == 4 * qc:
                        # this and later tiles read group-qc keys: K(qc)
                        # chunks must be fully emitted first
                        drain_until(f"k{qc}")
                    ST = ps.tile([128, 2, 512], F32, tag="st", bufs=2)
                    dq = max(0, kt * 128 - qlo)
                    for hi in range(2):
                        hslc = slice(hi * 64, (hi + 1) * 64)
                        nc.tensor.matmul(
                            ST[:, hi, dq:],
                            kt_sb[p][hslc, kt * 128:(kt + 1) * 128],
                            qt_sb[p][hslc, qlo + dq:qlo + 512],
                            start=True, stop=True)
                    PT = ptp.tile([128, 2, 512], BF16, tag="pt")
                    nc.scalar.activation(PT[:, :, dq:], ST[:, :, dq:],
                                         AF.Exp, scale=SCALE)
                    if dq or kt * 128 == qlo:  # diagonal tile
                        nc.vector.tensor_mul(
                            PT[:, :, dq:dq + 128],
                            PT[:, :, dq:dq + 128], mask_sb[:])
                    # filler sized to the exp/score engine-time difference
                    emit_filler((2 * (512 - dq) * 0.42 + 250)
                                * (2 if qc == 0 else 1))
                    return PT

                def need_score(p, upto):
                    while len(pts[p]) <= upto:
                        pts[p].append(score(p, len(pts[p])))

                def pv_chain(p, qt, on_act=False):
                    """P@V for query tile qt: O[q, 65] per head, col 64 is
                    the ones-column softmax denominator; then normalize into
                    ON and DMA-transpose into OT."""
                    qt_g = 4 * qc + qt
                    qts = slice(qt * 128, (qt + 1) * 128)
                    ON, OT = onot[qt]
                    epoch[0] += 1
                    Of = ps.tile([128, 512], F32, tag="oy", bufs=2)
                    O = Of[:, 0:130].rearrange("p (h e) -> p h e", e=65)
                    for hi in range(2):  # hi-sequential: same-bank groups
                        hd = (2 * p + hi) * 65
                        for kt in range(qt_g + 1):
                            nc.tensor.matmul(
                                O[:, hi, :], pts[p][kt][:, hi, qts],
                                vp_sb[:, kt, hd:hd + 65],
                                start=(kt == 0), stop=(kt == qt_g))
                    Ri = rip.tile([128, 2], F32, tag="ri")
                    with nc.allow_low_precision(reason="softmax recip"):
                        nc.vector.reciprocal(Ri[:, :], O[:, :, 64:65])
                    for hi in range(2):
                        if on_act:  # endgame: exp stream done, ACT is free
                            nc.scalar.mul(ON[:, p, hi * 64:(hi + 1) * 64],
                                          O[:, hi, 0:64], Ri[:, hi:hi + 1])
                        else:
                            nc.vector.tensor_scalar_mul(
                                ON[:, p, hi * 64:(hi + 1) * 64],
                                O[:, hi, 0:64], Ri[:, hi:hi + 1])
                    # [q, d] -> [d, q] via DMA XBAR transpose
                    nc.sync.dma_start(OT[:, p, :], ON[:, p, :], transpose=True)

                # p0 scores below the diagonal block row, plus a few p1
                # scores so the exp stream rides over the V' drain
                need_score(0, 4 * qc)
                need_score(1, min(2, ktmax - 1))
                # K(qc+1) queued only after the kt=4qc guard has fired, so
                # the guard never drains the NEXT group's K early
                filler_ch.extend(late_k)
                drain_until(f"v{qc}")  # V' must exist before P@V reads it
                # p0 diagonal scores + qt tail, with p1's early scores
                # interleaved so the exp stream never pauses
                for qt in range(4):
                    need_score(0, 4 * qc + qt)
                    need_score(1, min(qt, ktmax - 1))
                    pv_chain(0, qt)
                if qc < NQC - 1:
                    for qt in range(4):
                        need_score(1, 4 * qc + qt)
                        pv_chain(1, qt)
                        # defer: out-projection is PE work that fills the
                        # last chunks' exp-wait gaps
                        filler_yp.append(
                            (900, (lambda o=onot[qt][1], q=4 * qc + qt:
                                   out_proj(o, q)), epoch[0]))
                else:
                    # endgame: finish the exp stream first, then run the
                    # chain/normalize/transpose tail as one pipeline
                    need_score(1, ktmax - 1)
                    for qt in range(4):
                        pv_chain(1, qt, on_act=True)
                        if qt >= 2:
                            emit_filler(1800)
                    for qt in range(4):
                        out_proj(onot[qt][1], 4 * qc + qt,
                                 on_act=(qt >= 2), endgame=True)
                # group qc+1 must be fully projected before A(qc+1) emits
                drain_until(f"q{qc + 1}")
                if qc == NQC - 1:
                    drain_until(f"k{qc}")
                    drain_yp()

            wps = ps.tile([128, 128], F32, tag="oy", bufs=2)
            for i in range(26):
                nc.tensor.matmul(wps[:], wrm[:], wrm[:],
                                 start=(i == 0), stop=(i == 25))
            proj_group0()
            for c in range(NQC):
                filler_ch.extend(v_chains(c))
                late_k = qk_chain(c + 1, "k") if c + 1 < NQC else ()
                if c + 1 < NQC:
                    filler_ch.extend(qk_chain(c + 1, "q"))
                attention(c, late_k)

    if split_waits:
        _split_excess_waits(nc, max_waits=1)
    return nc


_NC = None


def kernel(x, Wq, Wk, Wv, Wo):
    global _NC
    if _NC is None:
        _NC = _build_nc()
    BF = ml_dtypes.bfloat16
    x = np.asarray(x, dtype=np.float32)
    Wq, Wk, Wv, Wo = (np.asarray(w, dtype=np.float32) for w in (Wq, Wk, Wv, Wo))

    tri = np.triu(np.ones((128, 128), dtype=np.float32))  # m[k,q] = k<=q
    in_maps = []
    for core in range(8):
        b, g = divmod(core, G)
        csl = slice(g * CL, (g + 1) * CL)
        in_maps.append({
            "xt": np.ascontiguousarray(x[b].T).astype(BF),
            "wq": np.ascontiguousarray(Wq[csl, :].T).astype(BF),
            "wk": np.ascontiguousarray(Wk[csl, :].T).astype(BF),
            "wv": np.ascontiguousarray(Wv[csl, :].T).astype(BF),
            "wo": np.ascontiguousarray(
                Wo[:, csl].T.reshape(2, 128, D).transpose(1, 0, 2)).astype(BF),
            "mask": np.ascontiguousarray(np.stack([tri, tri], axis=1)).astype(BF),
        })
    res = run_bass_kernel_spmd(_NC, in_maps, list(range(8)))
    y = np.empty((B, S, D), dtype=np.float32)
    for b in range(B):
        acc = np.asarray(res.results[4 * b]["y"]).astype(np.float32)
        for g in range(1, G):
            acc = acc + np.asarray(res.results[4 * b + g]["y"]).astype(np.float32)
        y[b] = acc
    return y


# revision 57
# speedup vs baseline: 1.0387x; 1.0387x over previous
"""Causal self-attention (B=2, S=2048, D=1024, H=16) on 8 NeuronCores.

Sharding (per spec hint): data-parallel over batch (2 groups of 4 cores),
tensor-parallel over heads within a group (4 heads / core). Each core
computes Q/K/V projections for its 4 heads, causal attention, and a partial
output projection through its slice of Wo. The 4 partial [2048, 1024]
outputs per batch are summed on the host (unsharding step).

Per-core kernel layout notes:
  - bf16 operands everywhere on the PE (fp32 PSUM accumulation): halves DMA
    traffic and allows arbitrary moving sizes at full PE rate, enabling
    exact causal trimming of the score matmuls.
  - Projections run k-chunk-major inside each 512-query group so the PE
    consumes x.T chunks as their DMAs land; attention for query chunk c-1
    is emitted between projection groups c-1 and c so the scalar engine's
    exp stream (the attention bottleneck) starts ~30us before projections
    finish.
  - Scores are computed transposed, ST[k, q]; P@V is computed with P as
    the stationary operand per 128-query tile, producing O[q, d] directly
    with a fused ones-column denominator (col 64 of each head's V block).
    Softmax normalize is then a per-partition tensor_scalar multiply, and
    the two heads of a pair pack side-by-side in the free dim. A DMA
    transpose (XBAR) turns the normalized [q, d] tile into the [d, q]
    stationary tile the out-projection needs - no PE transposes, no
    partition-crossing copies.
  - A single 8-bank PSUM pool (tag "big" 2-bank entries x3, tag "o"
    1-bank entries x2) is shared by projections, scores, P@V and the
    out-projection so all phases can be in flight at once.
"""

import numpy as np
import ml_dtypes

import concourse.bass as bass
import concourse.mybir as mybir
import concourse.tile as tile
from concourse.bass_utils import run_bass_kernel_spmd

F32 = mybir.dt.float32
BF16 = mybir.dt.bfloat16
AF = mybir.ActivationFunctionType

B, S, D, H = 2, 2048, 1024, 16
DH = D // H              # 64
HL = 4                   # heads per core
CL = HL * DH             # 256 channels per core
G = 4                    # cores per batch group
SCALE = DH ** -0.5       # 0.125
NQC = S // 512           # 4 q-chunks of 512
NKT = S // 128           # 16 key tiles of 128


def _split_excess_waits(nc, max_waits=1):
    """walrus in this toolchain rejects instructions carrying more than
    `max_waits` sem waits; split the excess onto preceding same-engine
    NoOps (sound: waits are monotone >= conditions hoisted earlier on
    the same engine)."""
    n_split = 0
    for f in nc.m.functions:
        for bb in f.blocks:
            out = []
            for inst in bb.instructions:
                si = inst.sync_info
                waits = list(si.on_wait) if si is not None and si.on_wait else []
                if len(waits) > max_waits:
                    head, keep = waits[:-max_waits], waits[-max_waits:]
                    for ci, start in enumerate(range(0, len(head), max_waits)):
                        nop = mybir.InstNoOp(
                            name=f"{inst.name}_wsplit{ci}",
                            sync_info=mybir.SyncInfo(
                                on_wait=head[start:start + max_waits],
                                on_update=[],
                            ),
                            engine=inst.engine,
                            bass_nofuse=True,
                        )
                        out.append(nop)
                        n_split += 1
                    si.on_wait = keep
                out.append(inst)
            if n_split:
                bb.instructions.clear()
                for i in out:
                    bb.instructions.append(i)
    return n_split


def _build_nc(split_waits=True):
    nc = bass.Bass()
    xt_d = nc.dram_tensor("xt", [D, S], BF16, kind="ExternalInput")
    wq_d = nc.dram_tensor("wq", [D, CL], BF16, kind="ExternalInput")
    wk_d = nc.dram_tensor("wk", [D, CL], BF16, kind="ExternalInput")
    wv_d = nc.dram_tensor("wv", [D, CL], BF16, kind="ExternalInput")
    wo_d = nc.dram_tensor("wo", [128, 2, D], BF16, kind="ExternalInput")
    mask_d = nc.dram_tensor("mask", [128, 2, 128], BF16, kind="ExternalInput")
    y_d = nc.dram_tensor("y", [S, D], BF16, kind="ExternalOutput")

    with tile.TileContext(nc) as tc:
        with (
            tc.tile_pool(name="persist", bufs=1) as pp,
            tc.tile_pool(name="ps", bufs=1, space="PSUM") as ps,
            tc.tile_pool(name="ptp", bufs=24) as ptp,
            tc.tile_pool(name="onot", bufs=4) as onp,
            tc.tile_pool(name="otp", bufs=14) as otp,
            tc.tile_pool(name="ysp", bufs=8) as ysp,
            tc.tile_pool(name="rip", bufs=4) as rip,
        ):
            # ---- persistent SBUF tensors -------------------------------
            xt_sb = pp.tile([128, 8, S], BF16)
            wq_sb = pp.tile([128, 8, CL], BF16)
            wk_sb = pp.tile([128, 8, CL], BF16)
            wv_sb = pp.tile([128, 8, CL], BF16)
            wo_sb = pp.tile([128, 2, D], BF16)
            mask_sb = pp.tile([128, 2, 128], BF16)
            qt_sb = [pp.tile([128, S], BF16, name=f"qt{p}", tag=f"qt{p}")
                     for p in range(2)]
            kt_sb = [pp.tile([128, S], BF16, name=f"kt{p}", tag=f"kt{p}")
                     for p in range(2)]
            # V' per key-tile: 4x[64 v-cols + 1 ones-col]
            vp_sb = pp.tile([128, NKT, 4 * 65], BF16)
            wrm = pp.tile([128, 128], BF16)
            nc.gpsimd.memset(wrm[:], 0.0)
            for hl in range(4):
                nc.vector.memset(vp_sb[:, :, hl * 65 + 64:hl * 65 + 65], 1.0)

            # ---- input DMAs (SP queue, arrival-ordered) ----------------
            # group 0 only touches xt cols 0:512 (its queries AND its V row
            # block), so stream those quarters first, weights interleaved.
            xt_r = xt_d.rearrange("(a p) s -> p a s", p=128)
            wq_r = wq_d.rearrange("(a p) m -> p a m", p=128)
            wk_r = wk_d.rearrange("(a p) m -> p a m", p=128)
            wv_r = wv_d.rearrange("(a p) m -> p a m", p=128)
            # few, large DMAs (each dma_start costs ~625ns on the serial
            # HWDGE descriptor engine) but with small first chunks so the
            # first projection chain starts at ~2us.
            for kk in range(0, 8, 2):
                kks = slice(kk, kk + 2)
                nc.sync.dma_start(wq_sb[:, kks, :], wq_r[:, kks, :])
                nc.sync.dma_start(wk_sb[:, kks, :], wk_r[:, kks, :])
                nc.sync.dma_start(xt_sb[:, kks, 0:512], xt_r[:, kks, 0:512])
            nc.sync.dma_start(mask_sb[:], mask_d[:, :, :])
            nc.sync.dma_start(xt_sb[:, :, 512:1024], xt_r[:, :, 512:1024])
            nc.sync.dma_start(wv_sb[:], wv_r[:])
            nc.sync.dma_start(wo_sb[:], wo_d[:, :, :])
            nc.sync.dma_start(xt_sb[:, :, 1024:1536], xt_r[:, :, 1024:1536])
            nc.sync.dma_start(xt_sb[:, :, 1536:2048], xt_r[:, :, 1536:2048])

            # ---- PE filler queue ---------------------------------------
            # Projection work for group c+1 is queued in small chunks and
            # emitted between attention score matmuls, so the PE has ready
            # work while it waits for the scalar engine's exp stream (the
            # attention bottleneck) to drain score tiles.
            # one strict-FIFO queue of projection-chain chunks (labels
            # mark drain barriers); a chain's chunks stay contiguous so at
            # most one chain is mid-flight on the "w" ring at any time
            filler_ch = []       # (est, fn, label)
            filler_yp = []       # deferred out-projections: lowest priority

            epoch = [0]
            debt = [0.0]  # carried-over filler budget (amortizes chunk
                          # granularity so the ST stream never lags ACT)

            def emit_filler(budget, reserve_yp=0):
                debt[0] += budget
                while True:
                    if filler_ch:
                        q = filler_ch
                    elif len(filler_yp) > reserve_yp \
                            and filler_yp[0][2] <= epoch[0] - 2:
                        q = filler_yp
                    else:
                        debt[0] = min(debt[0], 2000.0)
                        return
                    if q[0][0] > debt[0]:
                        return
                    item = q.pop(0)
                    item[1]()
                    debt[0] -= item[0]

            def drain_until(label):
                while any(it[2] == label for it in filler_ch):
                    filler_ch.pop(0)[1]()

            def drain_yp():
                while filler_yp:
                    filler_yp.pop(0)[1]()

            # ---- projection chains (tag rings: see module docstring) ---
            def qk_chain(c, which):
                """Full 8-step QK chain for group c as filler chunks."""
                cslc = slice(c * 512, (c + 1) * 512)
                w_sb = wq_sb if which == "q" else wk_sb
                dst = qt_sb if which == "q" else kt_sb
                box = {}

                def step(k, p):
                    def go():
                        if k == 0:
                            box[p] = ps.tile([128, 512], F32, tag="w",
                                             bufs=2, name=f"pj{which}{c}{p}")
                        nc.tensor.matmul(
                            box[p][:], w_sb[:, k, p * 128:(p + 1) * 128],
                            xt_sb[:, k, cslc], start=(k == 0), stop=(k == 7))
                        if k == 7:
                            nc.vector.tensor_copy(dst[p][:, cslc], box[p][:])
                    return go
                return [(214, step(k, p), f"{which}{c}")
                        for k in range(8) for p in range(2)]

            def v_chains(c):
                """V chains for group c (4 row-tiles, j-sequential)."""
                box = {}

                def step(j, kk):
                    def go():
                        pair, jj = divmod(j, 2)
                        if jj == 0 and kk == 0:
                            box[pair] = ps.tile([128, 2, 256], F32, tag="w",
                                                bufs=2, name=f"pjv{c}{pair}")
                        st = 4 * c + j
                        for k in range(kk, kk + 2):
                            nc.tensor.matmul(
                                box[pair][:, jj, :],
                                xt_sb[:, k, st * 128:(st + 1) * 128],
                                wv_sb[:, k, :], start=(k == 0), stop=(k == 7))
                        if jj == 1 and kk == 6:
                            nc.vector.tensor_copy(
                                vp_sb[:, 4 * c + 2 * pair:4 * c + 2 * pair + 2, :]
                                .rearrange("p s (h e) -> p (s h) e", e=65)
                                [:, :, 0:64],
                                box[pair][:]
                                .rearrange("p s (h d) -> p (s h) d", d=64))
                    return go
                return [(214, step(j, kk), f"v{c}")
                        for j in range(4) for kk in (0, 2, 4, 6)]

            # ---- group 0: Q then K chains (V rides the filler queue) ----
            def proj_group0():
                psq = ps.tile([128, 2, 512], F32, tag="st", bufs=2)
                psk = ps.tile([128, 2, 512], F32, tag="st", bufs=2)
                for k in range(8):
                    first, last = k == 0, k == 7
                    for p in range(2):
                        nc.tensor.matmul(
                            psq[:, p, :], wq_sb[:, k, p * 128:(p + 1) * 128],
                            xt_sb[:, k, 0:512], start=first, stop=last)
                    for p in range(2):
                        nc.tensor.matmul(
                            psk[:, p, :], wk_sb[:, k, p * 128:(p + 1) * 128],
                            xt_sb[:, k, 0:512], start=first, stop=last)
                for p in range(2):
                    nc.scalar.copy(qt_sb[p][:, 0:512], psq[:, p, :])
                    nc.vector.tensor_copy(kt_sb[p][:, 0:512], psk[:, p, :])

            # ---- attention + out-projection for query chunk qc ---------
            y_r = y_d.rearrange("(a p) d -> p a d", p=128)

            def out_proj(OT, qt_g, on_act=False, endgame=False):
                ysb = ysp.tile([128, D], BF16, tag="ys")
                if endgame:
                    # score ring is free by now: 2-bank paired tiles let the
                    # four tail out-projections overlap their copies
                    ypp = ps.tile([128, 2, 512], F32, tag="st", bufs=2)
                for nch in range(2):
                    if endgame:
                        yp = ypp[:, nch, :]
                    else:
                        yp = ps.tile([128, 512], F32, tag="oy", bufs=2)
                    for kp in range(2):
                        nc.tensor.matmul(
                            yp, OT[:, kp, :],
                            wo_sb[:, kp, nch * 512:(nch + 1) * 512],
                            start=(kp == 0), stop=(kp == 1))
                    nsl = slice(nch * 512, (nch + 1) * 512)
                    if on_act and nch:  # endgame: split copies DVE/ACT
                        nc.scalar.copy(ysb[:, nsl], yp)
                    else:
                        nc.vector.tensor_copy(ysb[:, nsl], yp)
                    if endgame:  # idle SP queue, per-half DMAs
                        nc.sync.dma_start(y_r[:, qt_g, nsl], ysb[:, nsl])
                if not endgame:
                    nc.gpsimd.dma_start(y_r[:, qt_g, :], ysb[:])

            def attention(qc, late_k=()):
                qlo = qc * 512
                ktmax = 4 * (qc + 1)
                onot = [(onp.tile([128, 2, 128], BF16, tag="on", name=f"on{qt}"),
                         otp.tile([128, 2, 128], BF16, tag="ot", name=f"ot{qt}"))
                        for qt in range(4)]
                pts = {0: [], 1: []}

                def score(p, kt):
                    if kt == 4 * qc:
                        # this and later tiles read group-qc keys: K(qc)
                        # chunks must be fully emitted first
                        drain_until(f"k{qc}")
                    ST = ps.tile([128, 2, 512], F32, tag="st", bufs=2)
                    dq = max(0, kt * 128 - qlo)
                    for hi in range(2):
                        hslc = slice(hi * 64, (hi + 1) * 64)
                        nc.tensor.matmul(
                            ST[:, hi, dq:],
                            kt_sb[p][hslc, kt * 128:(kt + 1) * 128],
                            qt_sb[p][hslc, qlo + dq:qlo + 512],
                            start=True, stop=True)
                    PT = ptp.tile([128, 2, 512], BF16, tag="pt")
                    nc.scalar.activation(PT[:, :, dq:], ST[:, :, dq:],
                                         AF.Exp, scale=SCALE)
                    if dq or kt * 128 == qlo:  # diagonal tile
                        nc.vector.tensor_mul(
                            PT[:, :, dq:dq + 128],
                            PT[:, :, dq:dq + 128], mask_sb[:])
                    # filler sized to the exp/score engine-time difference
                    emit_filler((2 * (512 - dq) * 0.42 + 250)
                                * (2 if qc == 0 else 1))
                    return PT

                def need_score(p, upto):
                    while len(pts[p]) <= upto:
                        pts[p].append(score(p, len(pts[p])))

                def pv_chain(p, qt, on_act=False):
                    """P@V for query tile qt: O[q, 65] per head, col 64 is
                    the ones-column softmax denominator; then normalize into
                    ON and DMA-transpose into OT."""
                    qt_g = 4 * qc + qt
                    qts = slice(qt * 128, (qt + 1) * 128)
                    ON, OT = onot[qt]
                    epoch[0] += 1
                    Of = ps.tile([128, 512], F32, tag="oy", bufs=2)
                    O = Of[:, 0:130].rearrange("p (h e) -> p h e", e=65)
                    for hi in range(2):  # hi-sequential: same-bank groups
                        hd = (2 * p + hi) * 65
                        for kt in range(qt_g + 1):
                            nc.tensor.matmul(
                                O[:, hi, :], pts[p][kt][:, hi, qts],
                                vp_sb[:, kt, hd:hd + 65],
                                start=(kt == 0), stop=(kt == qt_g))
                    Ri = rip.tile([128, 2], F32, tag="ri")
                    with nc.allow_low_precision(reason="softmax recip"):
                        nc.vector.reciprocal(Ri[:, :], O[:, :, 64:65])
                    for hi in range(2):
                        if on_act:  # endgame: exp stream done, ACT is free
                            nc.scalar.mul(ON[:, p, hi * 64:(hi + 1) * 64],
                                          O[:, hi, 0:64], Ri[:, hi:hi + 1])
                        else:
                            nc.vector.tensor_scalar_mul(
                                ON[:, p, hi * 64:(hi + 1) * 64],
                                O[:, hi, 0:64], Ri[:, hi:hi + 1])
                    # [q, d] -> [d, q] via DMA XBAR transpose
                    nc.sync.dma_start(OT[:, p, :], ON[:, p, :], transpose=True)

                # p0 scores below the diagonal block row, plus a few p1
                # scores so the exp stream rides over the V' drain
                need_score(0, 4 * qc)
                need_score(1, min(2, ktmax - 1))
                # K(qc+1) queued only after the kt=4qc guard has fired, so
                # the guard never drains the NEXT group's K early
                filler_ch.extend(late_k)
                drain_until(f"v{qc}")  # V' must exist before P@V reads it
                # p0 diagonal scores + qt tail, with p1's early scores
                # interleaved so the exp stream never pauses
                for qt in range(4):
                    need_score(0, 4 * qc + qt)
                    need_score(1, min(qt, ktmax - 1))
                    pv_chain(0, qt)
                if qc < NQC - 1:
                    for qt in range(4):
                        need_score(1, 4 * qc + qt)
                        pv_chain(1, qt)
                        # defer: out-projection is PE work that fills the
                        # last chunks' exp-wait gaps
                        filler_yp.append(
                            (900, (lambda o=onot[qt][1], q=4 * qc + qt:
                                   out_proj(o, q)), epoch[0]))
                else:
                    # endgame: finish the exp stream first, then run the
                    # chain/normalize/transpose tail as one pipeline
                    need_score(1, ktmax - 1)
                    for qt in range(4):
                        pv_chain(1, qt, on_act=True)
                        if qt >= 2:
                            emit_filler(1800)
                    for qt in range(4):
                        out_proj(onot[qt][1], 4 * qc + qt,
                                 on_act=(qt >= 2), endgame=True)
                # group qc+1 must be fully projected before A(qc+1) emits
                drain_until(f"q{qc + 1}")
                if qc == NQC - 1:
                    drain_until(f"k{qc}")
                    drain_yp()

            wps = ps.tile([128, 128], F32, tag="oy", bufs=2)
            for i in range(26):
                nc.tensor.matmul(wps[:], wrm[:], wrm[:],
                                 start=(i == 0), stop=(i == 25))
            proj_group0()
            for c in range(NQC):
                filler_ch.extend(v_chains(c))
                late_k = qk_chain(c + 1, "k") if c + 1 < NQC else ()
                if c + 1 < NQC:
                    filler_ch.extend(qk_chain(c + 1, "q"))
                attention(c, late_k)

    if split_waits:
        _split_excess_waits(nc, max_waits=1)
    return nc


_NC = None


def kernel(x, Wq, Wk, Wv, Wo):
    global _NC
    if _NC is None:
        _NC = _build_nc()
    BF = ml_dtypes.bfloat16
    x = np.asarray(x, dtype=np.float32)
    Wq, Wk, Wv, Wo = (np.asarray(w, dtype=np.float32) for w in (Wq, Wk, Wv, Wo))

    tri = np.triu(np.ones((128, 128), dtype=np.float32))  # m[k,q] = k<=q
    in_maps = []
    for core in range(8):
        b, g = divmod(core, G)
        csl = slice(g * CL, (g + 1) * CL)
        in_maps.append({
            "xt": np.ascontiguousarray(x[b].T).astype(BF),
            "wq": np.ascontiguousarray(Wq[csl, :].T).astype(BF),
            "wk": np.ascontiguousarray(Wk[csl, :].T).astype(BF),
            "wv": np.ascontiguousarray(Wv[csl, :].T).astype(BF),
            "wo": np.ascontiguousarray(
                Wo[:, csl].T.reshape(2, 128, D).transpose(1, 0, 2)).astype(BF),
            "mask": np.ascontiguousarray(np.stack([tri, tri], axis=1)).astype(BF),
        })
    res = run_bass_kernel_spmd(_NC, in_maps, list(range(8)))
    y = np.empty((B, S, D), dtype=np.float32)
    for b in range(B):
        acc = np.asarray(res.results[4 * b]["y"]).astype(np.float32)
        for g in range(1, G):
            acc = acc + np.asarray(res.results[4 * b + g]["y"]).astype(np.float32)
        y[b] = acc
    return y
